# revision 32
# baseline (speedup 1.0000x reference)
"""MicroStepDecoder TRN2 kernel (v3: row-tile staggered pipeline).

Math (equivalent to reference via causality/KV-cache):
  gather N=2048 rows -> h0 [N, D]; 5 decode steps of one llama layer,
  step t attends over cached K/V of steps 0..t; output[n, t] = h after step t.

Device strategy: data-parallel over 8 cores, 256 rows/core (2 row-tiles of
128).  All matmuls bf16; lhsT = transposed activations (PE transposes), rhs =
streamed bf16 weight chunks (gate/up: weights stationary, activations moving).
RoPE / ln scales / 1/sqrt(hd) folded into weights on host.

v3 structural change vs v2: the two row-tiles are software-pipelined so the
DVE/ACT/GPSIMD attention chain of one row-tile overlaps PE matmuls of the
other (QKV of rt1 hides attn(rt0); O-proj of rt0 hides attn(rt1)).  norm1 is
folded into the QKV psum copy-outs (ACT copy with per-partition rstd scale)
so xnT is a raw transpose of h and step boundaries have no serial
normalize->transpose chain.  QKV weight chunks are loaded once and reused by
both row-tiles; rt1's QKV copy-outs are deferred until after attn(rt0)'s exps
so the ACT queue never blocks the attention chain.
"""
import numpy as np
import ml_dtypes

import concourse.bass as bass
import concourse.bacc as bacc
import concourse.tile as tile
import concourse.mybir as mybir
from concourse.masks import make_identity
from concourse.bass import _add_dep_helper
from concourse.bass_utils import run_bass_kernel_spmd

F32 = mybir.dt.float32
F16 = mybir.dt.float16
BF16 = mybir.dt.bfloat16
AX = mybir.AxisListType
ALU = mybir.AluOpType
ACTF = mybir.ActivationFunctionType

D = 2048
DFF = 8192
HEADS = 32
KVH = 8
HD = 64
REP = HEADS // KVH
STEPS = 5
NCORES = 8
R = 256            # rows per core
RT = 2             # row tiles per core
KT = D // 128      # 16
EPS = 1e-6
THETA = 1e4
NP_W = ml_dtypes.bfloat16
SIM_SAFE = False   # replace Silu with Sigmoid*x for CoreSim runs
GP_ATTN = True     # offload e_j*V_j broadcast product to GPSIMD

_CACHE = {}


# ---------------------------------------------------------------- device views
def _q8h(ap):   # [128, 2048] -> [128, kv8, rep, hd]
    return ap.rearrange("p (kv r d) -> p kv r d", kv=KVH, r=REP)


def _kv8h(ap):  # [128, 512] -> [128, kv8, rep(bc), hd]
    a3 = ap.rearrange("p (kv d) -> p kv d", kv=KVH)
    return a3[:, :, None, :].broadcast_to((128, KVH, REP, HD))


def _hb8h(ap):  # [128, 32] -> [128, kv8, rep, hd(bc)]
    a3 = ap.rearrange("p (kv r) -> p kv r", kv=KVH)
    return a3[:, :, :, None].broadcast_to((128, KVH, REP, HD))


# ---------------------------------------------------------------- program
def _build_program():
    nc = bacc.Bacc("TRN2", target_bir_lowering=False, debug=False)

    h0_d = nc.dram_tensor("h0", [RT, 128, D], F32, kind="ExternalInput")
    qkv_d = nc.dram_tensor("wqkv", [STEPS, 24, 128, 2048], BF16, kind="ExternalInput")
    o_d = nc.dram_tensor("wo", [16, 128, 2048], BF16, kind="ExternalInput")
    o0_d = nc.dram_tensor("wo0", [4, 128, 2048], BF16, kind="ExternalInput")
    g_d = nc.dram_tensor("wg", [32, 128, 4096], BF16, kind="ExternalInput")
    u_d = nc.dram_tensor("wu", [32, 128, 4096], BF16, kind="ExternalInput")
    d_d = nc.dram_tensor("wd", [32, 128, 4096], BF16, kind="ExternalInput")
    out_d = nc.dram_tensor("out", [STEPS, RT, 128, D], F32, kind="ExternalOutput")

    with tile.TileContext(nc) as tc:
        with (
            tc.tile_pool(name="per", bufs=1) as per,       # persistent
            tc.tile_pool(name="scr", bufs=2) as scr,       # [128, D] bf16 scratch
            tc.tile_pool(name="asc", bufs=3) as asc,
            tc.tile_pool(name="avp", bufs=4) as avp,
            tc.tile_pool(name="wts", bufs=4) as wts,
            tc.tile_pool(name="sm", bufs=12) as sm,        # small tiles
            tc.tile_pool(name="ps_mm", bufs=4, space=bass.MemorySpace.PSUM) as ps_mm,
            tc.tile_pool(name="ps_gu", bufs=2, space=bass.MemorySpace.PSUM) as ps_gu,
            tc.tile_pool(name="ps_tp", bufs=2, space=bass.MemorySpace.PSUM) as ps_tp,
        ):
            ident_b = per.tile([128, 128], BF16, tag="ident_b", name="ident_b")
            make_identity(nc, ident_b[:])
            ident_f = per.tile([128, 128], F32, tag="ident_f", name="ident_f")
            make_identity(nc, ident_f[:])
            ADT = BF16
            eps_t = per.tile([128, 1], F32, tag="eps", name="eps")
            nc.vector.memset(eps_t[:], EPS)

            h = [per.tile([128, D], F32, tag=f"h{rt}", name=f"h{rt}") for rt in range(RT)]
            Q = [per.tile([128, 2048], ADT, tag=f"q{rt}", name=f"q{rt}")
                 for rt in range(RT)]
            # K cache stored rep-EXPANDED to all 32 heads so attention's
            # q*k products are plain 2D ops (fast DVE mode)
            Ke = [[per.tile([128, 2048], ADT, tag=f"ke{t}_{rt}", name=f"ke{t}_{rt}")
                   for rt in range(RT)] for t in range(STEPS)]
            Vc = [[per.tile([128, 512], ADT, tag=f"vc{t}_{rt}", name=f"vc{t}_{rt}")
                   for rt in range(RT)] for t in range(STEPS)]
            oa = [per.tile([128, 2048], ADT, tag=f"oa{rt}", name=f"oa{rt}")
                  for rt in range(RT)]
            # per-rt transposed 128-col tiles, shared between the norm1
            # transpose (QKV lhsT) and the attn-out transpose (O lhsT) --
            # their lifetimes are disjoint within a step
            aT = [per.tile([128, KT, 128], BF16, tag=f"aT{rt}", name=f"aT{rt}")
                  for rt in range(RT)]
            x2T = per.tile([128, KT, R], BF16, tag="x2T", name="x2T")
            mT = per.tile([128, 32, R], BF16, tag="mT", name="mT")

            for rt in range(RT):
                nc.sync.dma_start(h[rt][:], h0_d[:][rt])

            dum = per.tile([128, 1], F32, tag="dum", name="dum")
            nc.vector.memset(dum[:], 1.0)
            dumo = per.tile([128, 1], F32, tag="dumo", name="dumo")

            def act_preload(func):
                # dummy activation so the ACT table load happens off the
                # critical path (during a matmul-heavy phase)
                nc.scalar.activation(dumo[:], dum[:], func=func)

            def rstd_of(rt, parts):
                # 1/rms of h[rt]; parts = 4 chunked sum-of-squares partials
                ssq = sm.tile([128, 1], F32, tag="ssq", name="ssq")
                if parts is not None:
                    p01 = sm.tile([128, 1], F32, tag="p01", name="p01")
                    nc.vector.tensor_add(p01[:], parts[rt][0][:], parts[rt][1][:])
                    p23 = sm.tile([128, 1], F32, tag="p23", name="p23")
                    nc.vector.tensor_add(p23[:], parts[rt][2][:], parts[rt][3][:])
                    nc.vector.tensor_add(ssq[:], p01[:], p23[:])
                else:
                    junk = scr.tile([128, D], BF16, tag="junk", name="junk")
                    nc.scalar.activation(junk[:], h[rt][:], func=ACTF.Square,
                                         accum_out=ssq[:])
                sd = sm.tile([128, 1], F32, tag="sd", name="sd")
                nc.scalar.activation(sd[:], ssq[:], func=ACTF.Sqrt,
                                     scale=1.0 / D, bias=eps_t[:])
                rstd = sm.tile([128, 1], F32, tag="rstd", name="rstd")
                nc.vector.reciprocal(rstd[:], sd[:])
                return rstd

            def tp_h(rt, ks):
                # raw transpose of fp32 residual chunk -> aT[rt] (bf16)
                for k in ks:
                    tp = ps_tp.tile([128, 128], F32, tag="tp", name="tp")
                    nc.tensor.transpose(
                        tp[:], h[rt][:, k * 128:(k + 1) * 128], ident_f[:])
                    nc.vector.tensor_copy(aT[rt][:, k, :], tp[:])

            def drain_add(rt, ch, pt, parts):
                # h[rt] chunk += psum; optionally compute the chunk's
                # sum-of-squares for the next norm while matmuls continue
                nc.vector.tensor_add(
                    h[rt][:, ch * 512:(ch + 1) * 512],
                    h[rt][:, ch * 512:(ch + 1) * 512], pt[:])
                if parts is not None:
                    junk = scr.tile([128, 512], BF16, tag="junk", name="jk5")
                    part = sm.tile([128, 1], F32, tag="sqp", name="sqp")
                    nc.scalar.activation(junk[:], h[rt][:, ch * 512:(ch + 1) * 512],
                                         func=ACTF.Square, accum_out=part[:])
                    parts[rt].append(part)

            dn_parts = None
            for t in range(STEPS):
                rstds = {}

                def qkv_phase():
                    # Q chunks first so attention can start mid-phase.
                    # Each weight chunk is loaded ONCE and used by both row
                    # tiles (halves the region's HBM demand, which would
                    # otherwise outrun DMA at ~590 GB/s and starve the PE).
                    order = (0, 1) if t == 0 else (2, 3, 4, 5, 0, 1)
                    for ch in order:
                        pq = [ps_mm.tile([128, 512], F32, tag="mm",
                                         name=f"pq{_rt}") for _rt in range(RT)]
                        for kg in range(4):
                            w = wts.tile([128, 4096], BF16, tag="w", name="wqkv")
                            nc.sync.dma_start(w[:, :2048], qkv_d[:][t, ch * 4 + kg])
                            for i in range(4):
                                k = kg * 4 + i
                                for rt in range(RT):
                                    nc.tensor.matmul(
                                        pq[rt][:], aT[rt][:, k, :],
                                        w[:, i * 512:(i + 1) * 512],
                                        start=(k == 0), stop=(k == KT - 1))
                        for rt in range(RT):
                            qkv_copy(rt, ch, pq[rt])

                def qkv_copy(rt, ch, pq):
                    # psum -> SBUF, rms-norm rstd folded into the scale
                    if ch == 0:
                        # store K rep-expanded to all heads: psum -> rep-0
                        # slice on ACT (scaled), then replicate to the other
                        # rep slots with small SBUF->SBUF DMAs
                        ke4 = Ke[t][rt][:].rearrange(
                            "p (kv r d) -> p kv r d", kv=KVH, r=REP)
                        nc.scalar.activation(
                            ke4[:, :, 0, :],
                            pq[:].rearrange("p (kv d) -> p kv d", kv=KVH),
                            func=ACTF.Copy, scale=rstds[rt][:])
                        for r in range(1, REP):
                            nc.sync.dma_start(ke4[:, :, r, :], ke4[:, :, 0, :])
                    elif ch == 1:
                        nc.scalar.activation(Vc[t][rt][:], pq[:],
                                             func=ACTF.Copy, scale=rstds[rt][:])
                    else:
                        part = ch - 2
                        dst = Q[rt][:, part * 512:(part + 1) * 512]
                        nc.scalar.activation(dst, pq[:], func=ACTF.Copy,
                                             scale=rstds[rt][:])

                def attn_rt(rt):
                    # t>=1 only (t=0 skips attention entirely via the wo0
                    # shortcut: softmax over one key == V).  Full-width
                    # [128,2048] ops: fewer cross-engine hops per step.
                    eng = nc.gpsimd if GP_ATTN else nc.vector
                    ejs, avs = [], []
                    oah = oa[rt]
                    for j in range(t + 1):
                        ascr = asc.tile([128, 2048], ADT, tag="ascr",
                                        name="ascr")
                        nc.vector.tensor_tensor(
                            ascr[:], Q[rt][:], Ke[j][rt][:], op=ALU.mult)
                        sc = sm.tile([128, 32], F32, tag="sc", name="sc")
                        nc.vector.tensor_reduce(
                            sc[:],
                            ascr[:].rearrange("p (h d) -> p h d", h=32),
                            axis=AX.X, op=ALU.add)
                        ej = sm.tile([128, 32], F32, tag="ej", name="ej")
                        nc.scalar.activation(ej[:], sc[:], func=ACTF.Exp)
                        ejs.append(ej)
                        if j == 0:
                            eng.tensor_tensor(
                                _q8h(oah[:]), _hb8h(ej[:]),
                                _kv8h(Vc[j][rt][:]), op=ALU.mult)
                        else:
                            av = avp.tile([128, 2048], ADT, tag="av",
                                          name="av")
                            eng.tensor_tensor(
                                _q8h(av[:]), _hb8h(ej[:]),
                                _kv8h(Vc[j][rt][:]), op=ALU.mult)
                            avs.append(av)
                    den = sm.tile([128, 32], F32, tag="den", name="den")
                    nc.vector.tensor_add(den[:], ejs[0][:], ejs[1][:])
                    for ej in ejs[2:]:
                        nc.vector.tensor_add(den[:], den[:], ej[:])
                    rec = sm.tile([128, 32], F32, tag="rec", name="rec")
                    nc.vector.reciprocal(rec[:], den[:])
                    for av in avs:
                        nc.vector.tensor_add(oah[:], oah[:], av[:])
                    nc.vector.tensor_tensor(
                        _q8h(oah[:]), _hb8h(rec[:]),
                        _q8h(oah[:]), op=ALU.mult)

                def o_tp_rt(rt):
                    if t == 0:
                        # aT := Vc^T (4 k-tiles); O uses the rep-summed wo0
                        for k in range(4):
                            tp = ps_tp.tile([128, 128], ADT, tag="tp", name="tp")
                            nc.tensor.transpose(
                                tp[:], Vc[0][rt][:, k * 128:(k + 1) * 128],
                                ident_b[:])
                            nc.vector.tensor_copy(aT[rt][:, k, :], tp[:])
                        return
                    for k in range(KT):
                        tp = ps_tp.tile([128, 128], ADT, tag="tp", name="tp")
                        nc.tensor.transpose(
                            tp[:], oa[rt][:, k * 128:(k + 1) * 128],
                            ident_b[:])
                        nc.vector.tensor_copy(aT[rt][:, k, :], tp[:])

                def o_phase_t0(o_parts):
                    # t=0: oa == V, so O contracts V^T against rep-summed wo0;
                    # cheap enough to run joint over row tiles
                    for ch in range(4):
                        po = [ps_mm.tile([128, 512], F32, tag="mm",
                                         name=f"po{_rt}") for _rt in range(RT)]
                        w = wts.tile([128, 4096], BF16, tag="w", name="w")
                        nc.sync.dma_start(w[:, :2048], o0_d[:][ch])
                        for k in range(4):
                            for rt in range(RT):
                                nc.tensor.matmul(
                                    po[rt][:], aT[rt][:, k, :],
                                    w[:, k * 512:(k + 1) * 512],
                                    start=(k == 0), stop=(k == 3))
                        for rt in range(RT):
                            drain_add(rt, ch, po[rt], o_parts)

                def o_mm_rt(rt, po, drains=None):
                    # per-row-tile O so attn(rt1) hides under O(rt0) matmuls;
                    # weights double-loaded (affordable: QKV is shared now)
                    for ch in range(4):
                        po[ch] = ps_mm.tile([128, 512], F32, tag="mm",
                                            name=f"po{ch}")
                        for kg in range(4):
                            w = wts.tile([128, 4096], BF16, tag="w", name="w")
                            nc.sync.dma_start(w[:, :2048], o_d[:][ch * 4 + kg])
                            for i in range(4):
                                k = kg * 4 + i
                                nc.tensor.matmul(
                                    po[ch][:], aT[rt][:, k, :],
                                    w[:, i * 512:(i + 1) * 512],
                                    start=(k == 0), stop=(k == KT - 1))
                        if drains is not None:
                            drain_add(rt, ch, po[ch], drains)

                x2s = {}

                def norm2_mul(rt, o_parts):
                    rstd = rstd_of(rt, o_parts)
                    x = scr.tile([128, D], BF16, tag="scr", name=f"x2_{rt}")
                    # chunked so the transposes can pipeline behind it
                    for c in range(4):
                        nc.scalar.activation(x[:, c * 512:(c + 1) * 512],
                                             h[rt][:, c * 512:(c + 1) * 512],
                                             func=ACTF.Copy, scale=rstd[:])
                    x2s[rt] = x

                def norm2_tp(rt):
                    x = x2s[rt]
                    for k in range(KT):
                        tp = ps_tp.tile([128, 128], BF16, tag="tp", name="tp")
                        nc.tensor.transpose(
                            tp[:], x[:, k * 128:(k + 1) * 128], ident_b[:])
                        nc.vector.tensor_copy(
                            x2T[:, k, rt * 128:(rt + 1) * 128], tp[:])

                # ---- norm1 rstd + (transposes done during prev D) + QKV ----
                for rt in range(RT):
                    rstds[rt] = rstd_of(rt, dn_parts)
                    if t == 0:
                        tp_h(rt, range(KT))
                if t >= 1:
                    act_preload(ACTF.Exp)
                qkv_phase()

                # ---- attention + O-proj, row-tile staggered ----
                o_parts = [[] for _ in range(RT)]
                if t == 0:
                    o_tp_rt(0)
                    o_tp_rt(1)
                    o_phase_t0(o_parts)
                else:
                    po0, po1 = {}, {}
                    attn_rt(0)      # overlaps QKV's K/V chunk tail on PE
                    o_tp_rt(0)
                    o_mm_rt(0, po0)
                    attn_rt(1)      # overlaps O(rt0) matmuls on PE
                    for ch in range(4):
                        drain_add(0, ch, po0[ch], o_parts)
                    norm2_mul(0, o_parts)
                    o_tp_rt(1)
                    o_mm_rt(1, po1, drains=o_parts)
                if t == 0:
                    norm2_mul(0, o_parts)
                norm2_mul(1, o_parts)
                norm2_tp(0)
                norm2_tp(1)
                act_preload(ACTF.Sigmoid if SIM_SAFE else ACTF.Silu)

                # ---- MLP in two ff halves: gate/up -> mT, then down ----
                dn_parts = [[] for _ in range(RT)] if t < STEPS - 1 else None
                for half in range(2):
                    for pr in range(16 * half, 16 * (half + 1)):
                        wg = wts.tile([128, 4096], BF16, tag="w", name="wgt")
                        nc.sync.dma_start(wg[:], g_d[:][pr])
                        wu = wts.tile([128, 4096], BF16, tag="w", name="wut")
                        nc.sync.dma_start(wu[:], u_d[:][pr])
                        for mgi in range(2):
                            mloc = (pr * 2 + mgi) - 32 * half
                            pg = ps_gu.tile([128, R], F32, tag="gu", name="pg")
                            for k in range(KT):
                                c = (mgi * KT + k) * 128
                                nc.tensor.matmul(
                                    pg[:], wg[:, c:c + 128], x2T[:, k, :],
                                    start=(k == 0), stop=(k == KT - 1))
                            pu = ps_gu.tile([128, R], F32, tag="gu", name="pu")
                            for k in range(KT):
                                c = (mgi * KT + k) * 128
                                nc.tensor.matmul(
                                    pu[:], wu[:, c:c + 128], x2T[:, k, :],
                                    start=(k == 0), stop=(k == KT - 1))
                            sg = sm.tile([128, R], BF16, tag="sg", name="sg")
                            if SIM_SAFE:
                                # CoreSim lacks Silu; silu(x) = x * sigmoid(x)
                                nc.scalar.activation(sg[:], pg[:], func=ACTF.Sigmoid)
                                tmp = asc.tile([128, R], F32, tag="sgt", name="sgt")
                                nc.vector.tensor_tensor(
                                    tmp[:], sg[:], pg[:], op=ALU.mult)
                                nc.vector.tensor_tensor(
                                    mT[:, mloc, :], tmp[:], pu[:], op=ALU.mult)
                            else:
                                nc.scalar.activation(sg[:], pg[:], func=ACTF.Silu)
                                nc.vector.tensor_tensor(
                                    mT[:, mloc, :], sg[:], pu[:], op=ALU.mult)
                    for ch in range(4):
                        pd_ = [ps_mm.tile([128, 512], F32, tag="mm", name=f"pd{_rt}")
                               for _rt in range(RT)]
                        for kfg in range(4 * half, 4 * (half + 1)):
                            w = wts.tile([128, 4096], BF16, tag="w", name="wdt")
                            nc.sync.dma_start(w[:], d_d[:][ch * 8 + kfg])
                            for i in range(8):
                                kf = kfg * 8 + i
                                kfl = kf - 32 * half
                                for rt in range(RT):
                                    nc.tensor.matmul(
                                        pd_[rt][:],
                                        mT[:, kfl, rt * 128:(rt + 1) * 128],
                                        w[:, i * 512:(i + 1) * 512],
                                        start=(kf == 32 * half),
                                        stop=(kf == 32 * half + 31))
                        for rt in range(RT):
                            drain_add(rt, ch, pd_[rt],
                                      dn_parts if half == 1 else None)
                            if half == 1 and t < STEPS - 1:
                                # next step's norm1 transposes, interleaved
                                # with the remaining down matmuls
                                tp_h(rt, range(ch * 4, ch * 4 + 4))

                # ---- store step output ----
                for rt in range(RT):
                    nc.gpsimd.dma_start(out_d[:][t, rt], h[rt][:])

    nc.compile()
    return nc


# ---------------------------------------------------------------- host prep
def _rope_cs(t):
    inv = 1.0 / (THETA ** (np.arange(0, HD, 2, dtype=np.float64) / HD))
    emb = np.concatenate([t * inv, t * inv])
    return np.cos(emb), np.sin(emb)


def _rope_cols(w, t, nheads):
    # w: [D, nheads*HD] fp; returns rope'd version for position t
    w3 = w.reshape(D, nheads, HD)
    cos, sin = _rope_cs(t)
    wrot = np.concatenate([-w3[:, :, HD // 2:], w3[:, :, :HD // 2]], axis=2)
    return (w3 * cos[None, None, :] + wrot * sin[None, None, :]).reshape(D, nheads * HD)


def _pack_rhs(w, n_ch, n_kg):
    # w [K, n_ch*512]; chunks (ch, kg): [128, 4*512]; kg covers 4 k-tiles
    kt = w.shape[0] // 128
    A = w.reshape(n_kg, kt // n_kg, 128, n_ch, 512)
    return np.ascontiguousarray(A.transpose(3, 0, 2, 1, 4)).reshape(
        n_ch * n_kg, 128, (kt // n_kg) * 512)


def _pack_lhs_gu(w):
    # w [D, DFF] -> [32 pairs][128, (mgi 2, k 16, 128)]
    B = w.reshape(KT, 128, 32, 2, 128)
    return np.ascontiguousarray(B.transpose(2, 1, 3, 0, 4)).reshape(32, 128, 4096)


def _pack_rhs_dn(w):
    # w [DFF, D] -> chunks (ch 4, kfg 8): [128, (i 8, 512)]
    C = w.reshape(8, 8, 128, 4, 512)
    return np.ascontiguousarray(C.transpose(3, 0, 2, 1, 4)).reshape(32, 128, 4096)


def _gather_indices(comp_seq_lens, inst_lens):
    seqs = np.asarray(comp_seq_lens)
    insts = np.asarray(inst_lens)
    idx, off = [], 0
    for s, i in zip(seqs, insts):
        s, i = int(s), int(i)
        idx.append(np.arange(off + i - 1, off + s - 1))
        off += s
    return np.concatenate(idx)


def _prep_inputs(hidden_states, comp_seq_lens, inst_lens, w_q, w_k, w_v, w_o,
                 ln1_w, ln2_w, w_gate, w_up, w_down):
    idx = _gather_indices(comp_seq_lens, inst_lens)
    h0 = np.asarray(hidden_states, np.float32)[0, idx]          # [N, D]
    N = h0.shape[0]
    assert N == NCORES * R, f"expected {NCORES*R} rows, got {N}"

    ln1 = np.asarray(ln1_w, np.float64)
    ln2 = np.asarray(ln2_w, np.float64)
    wq_e = np.asarray(w_q, np.float64) * ln1[:, None] * (HD ** -0.5)
    wk_e = np.asarray(w_k, np.float64) * ln1[:, None]
    wv_e = np.asarray(w_v, np.float64) * ln1[:, None]
    wg_e = np.asarray(w_gate, np.float64) * ln2[:, None]
    wu_e = np.asarray(w_up, np.float64) * ln2[:, None]

    qkv_pack = np.empty((STEPS, 24, 128, 2048), NP_W)
    for t in range(STEPS):
        wq_t = _rope_cols(wq_e, t, HEADS)
        wk_t = _rope_cols(wk_e, t, KVH)
        qkv = np.concatenate([wk_t, wv_e, wq_t], axis=1)  # K, V, Q order
        qkv_pack[t] = _pack_rhs(qkv, 6, 4).astype(NP_W)

    # t=0 shortcut: attention output == V (rep-broadcast), so O-proj
    # contracts V [*,512] against the rep-summed Wo rows
    wo64 = np.asarray(w_o, np.float64)
    wo0 = wo64.reshape(KVH, REP, HD, D).sum(axis=1).reshape(KVH * HD, D)

    weights = {
        "wqkv": qkv_pack,
        "wo": _pack_rhs(wo64, 4, 4).astype(NP_W),
        "wo0": _pack_rhs(wo0, 4, 1).astype(NP_W),
        "wg": _pack_lhs_gu(wg_e).astype(NP_W),
        "wu": _pack_lhs_gu(wu_e).astype(NP_W),
        "wd": _pack_rhs_dn(np.asarray(w_down, np.float64)).astype(NP_W),
    }
    h0_cores = h0.reshape(NCORES, RT, 128, D)
    return weights, h0_cores


def kernel(**inputs):
    weights, h0_cores = _prep_inputs(**inputs)

    if "nc" not in _CACHE:
        _CACHE["nc"] = _build_program()
    nc = _CACHE["nc"]

    in_maps = [dict(weights, h0=np.ascontiguousarray(h0_cores[c]))
               for c in range(NCORES)]
    res = run_bass_kernel_spmd(nc, in_maps, core_ids=list(range(NCORES)))
    _CACHE["last_results"] = res

    outs = []
    for c in range(NCORES):
        o = res.results[c]["out"]                  # [5, RT, 128, D]
        outs.append(o.reshape(STEPS, R, D).transpose(1, 0, 2))
    return np.concatenate(outs, axis=0)            # [N, 5, D]


# revision 33
# speedup vs baseline: 1.0322x; 1.0322x over previous
"""MicroStepDecoder TRN2 kernel (v3: row-tile staggered pipeline).

Math (equivalent to reference via causality/KV-cache):
  gather N=2048 rows -> h0 [N, D]; 5 decode steps of one llama layer,
  step t attends over cached K/V of steps 0..t; output[n, t] = h after step t.

Device strategy: data-parallel over 8 cores, 256 rows/core (2 row-tiles of
128).  All matmuls bf16; lhsT = transposed activations (PE transposes), rhs =
streamed bf16 weight chunks (gate/up: weights stationary, activations moving).
RoPE / ln scales / 1/sqrt(hd) folded into weights on host.

v3 structural change vs v2: the two row-tiles are software-pipelined so the
DVE/ACT/GPSIMD attention chain of one row-tile overlaps PE matmuls of the
other (QKV of rt1 hides attn(rt0); O-proj of rt0 hides attn(rt1)).  norm1 is
folded into the QKV psum copy-outs (ACT copy with per-partition rstd scale)
so xnT is a raw transpose of h and step boundaries have no serial
normalize->transpose chain.  QKV weight chunks are loaded once and reused by
both row-tiles; rt1's QKV copy-outs are deferred until after attn(rt0)'s exps
so the ACT queue never blocks the attention chain.
"""
import numpy as np
import ml_dtypes

import concourse.bass as bass
import concourse.bacc as bacc
import concourse.tile as tile
import concourse.mybir as mybir
from concourse.masks import make_identity
from concourse.bass import _add_dep_helper
from concourse.bass_utils import run_bass_kernel_spmd

F32 = mybir.dt.float32
F16 = mybir.dt.float16
BF16 = mybir.dt.bfloat16
AX = mybir.AxisListType
ALU = mybir.AluOpType
ACTF = mybir.ActivationFunctionType

D = 2048
DFF = 8192
HEADS = 32
KVH = 8
HD = 64
REP = HEADS // KVH
STEPS = 5
NCORES = 8
R = 256            # rows per core
RT = 2             # row tiles per core
KT = D // 128      # 16
EPS = 1e-6
THETA = 1e4
NP_W = ml_dtypes.bfloat16
SIM_SAFE = False   # replace Silu with Sigmoid*x for CoreSim runs
GP_ATTN = True     # offload e_j*V_j broadcast product to GPSIMD

_CACHE = {}


# ---------------------------------------------------------------- device views
def _q4h(ap):   # [128, 1024] -> [128, kv4, rep, hd]
    return ap.rearrange("p (kv r d) -> p kv r d", kv=KVH // 2, r=REP)


def _kv4h(ap):  # [128, 256] -> [128, kv4, rep(bc), hd]
    a3 = ap.rearrange("p (kv d) -> p kv d", kv=KVH // 2)
    return a3[:, :, None, :].broadcast_to((128, KVH // 2, REP, HD))


def _hb4h(ap):  # [128, 16] -> [128, kv4, rep, hd(bc)]
    a3 = ap.rearrange("p (kv r) -> p kv r", kv=KVH // 2)
    return a3[:, :, :, None].broadcast_to((128, KVH // 2, REP, HD))


# ---------------------------------------------------------------- program
def _build_program():
    nc = bacc.Bacc("TRN2", target_bir_lowering=False, debug=False)

    h0_d = nc.dram_tensor("h0", [RT, 128, D], F32, kind="ExternalInput")
    qkv_d = nc.dram_tensor("wqkv", [STEPS, 24, 128, 2048], BF16, kind="ExternalInput")
    o_d = nc.dram_tensor("wo", [16, 128, 2048], BF16, kind="ExternalInput")
    o0_d = nc.dram_tensor("wo0", [4, 128, 2048], BF16, kind="ExternalInput")
    g_d = nc.dram_tensor("wg", [32, 128, 4096], BF16, kind="ExternalInput")
    u_d = nc.dram_tensor("wu", [32, 128, 4096], BF16, kind="ExternalInput")
    d_d = nc.dram_tensor("wd", [32, 128, 4096], BF16, kind="ExternalInput")
    out_d = nc.dram_tensor("out", [STEPS, RT, 128, D], F32, kind="ExternalOutput")

    with tile.TileContext(nc) as tc:
        with (
            tc.tile_pool(name="per", bufs=1) as per,       # persistent
            tc.tile_pool(name="scr", bufs=2) as scr,       # [128, D] bf16 scratch
            tc.tile_pool(name="asc", bufs=3) as asc,
            tc.tile_pool(name="avp", bufs=8) as avp,
            tc.tile_pool(name="wts", bufs=4) as wts,
            tc.tile_pool(name="sm", bufs=12) as sm,        # small tiles
            tc.tile_pool(name="ps_mm", bufs=4, space=bass.MemorySpace.PSUM) as ps_mm,
            tc.tile_pool(name="ps_gu", bufs=2, space=bass.MemorySpace.PSUM) as ps_gu,
            tc.tile_pool(name="ps_tp", bufs=2, space=bass.MemorySpace.PSUM) as ps_tp,
        ):
            ident_b = per.tile([128, 128], BF16, tag="ident_b", name="ident_b")
            make_identity(nc, ident_b[:])
            ident_f = per.tile([128, 128], F32, tag="ident_f", name="ident_f")
            make_identity(nc, ident_f[:])
            ADT = BF16
            eps_t = per.tile([128, 1], F32, tag="eps", name="eps")
            nc.vector.memset(eps_t[:], EPS)

            h = [per.tile([128, D], F32, tag=f"h{rt}", name=f"h{rt}") for rt in range(RT)]
            Q = [[per.tile([128, 1024], ADT, tag=f"q{rt}_{hf}", name=f"q{rt}_{hf}")
                  for hf in range(2)] for rt in range(RT)]
            # K cache stored rep-EXPANDED to all 32 heads so attention's
            # q*k products are plain 2D ops (fast DVE mode)
            Ke = [[per.tile([128, 2048], ADT, tag=f"ke{t}_{rt}", name=f"ke{t}_{rt}")
                   for rt in range(RT)] for t in range(STEPS)]
            Vc = [[per.tile([128, 512], ADT, tag=f"vc{t}_{rt}", name=f"vc{t}_{rt}")
                   for rt in range(RT)] for t in range(STEPS)]
            oa = [[per.tile([128, 1024], ADT, tag=f"oa{rt}_{hf}", name=f"oa{rt}_{hf}")
                   for hf in range(2)] for rt in range(RT)]
            # per-rt transposed 128-col tiles, shared between the norm1
            # transpose (QKV lhsT) and the attn-out transpose (O lhsT) --
            # their lifetimes are disjoint within a step
            aT = [per.tile([128, KT, 128], BF16, tag=f"aT{rt}", name=f"aT{rt}")
                  for rt in range(RT)]
            x2T = per.tile([128, KT, R], BF16, tag="x2T", name="x2T")
            mT = per.tile([128, 32, R], BF16, tag="mT", name="mT")

            for rt in range(RT):
                nc.sync.dma_start(h[rt][:], h0_d[:][rt])

            dum = per.tile([128, 1], F32, tag="dum", name="dum")
            nc.vector.memset(dum[:], 1.0)
            dumo = per.tile([128, 1], F32, tag="dumo", name="dumo")

            def act_preload(func):
                # dummy activation so the ACT table load happens off the
                # critical path (during a matmul-heavy phase)
                nc.scalar.activation(dumo[:], dum[:], func=func)

            def rstd_of(rt, parts):
                # 1/rms of h[rt]; parts = 4 chunked sum-of-squares partials
                ssq = sm.tile([128, 1], F32, tag="ssq", name="ssq")
                if parts is not None:
                    p01 = sm.tile([128, 1], F32, tag="p01", name="p01")
                    nc.vector.tensor_add(p01[:], parts[rt][0][:], parts[rt][1][:])
                    p23 = sm.tile([128, 1], F32, tag="p23", name="p23")
                    nc.vector.tensor_add(p23[:], parts[rt][2][:], parts[rt][3][:])
                    nc.vector.tensor_add(ssq[:], p01[:], p23[:])
                else:
                    junk = scr.tile([128, D], BF16, tag="junk", name="junk")
                    nc.scalar.activation(junk[:], h[rt][:], func=ACTF.Square,
                                         accum_out=ssq[:])
                sd = sm.tile([128, 1], F32, tag="sd", name="sd")
                nc.scalar.activation(sd[:], ssq[:], func=ACTF.Sqrt,
                                     scale=1.0 / D, bias=eps_t[:])
                rstd = sm.tile([128, 1], F32, tag="rstd", name="rstd")
                nc.vector.reciprocal(rstd[:], sd[:])
                return rstd

            def tp_h(rt, ks):
                # raw transpose of fp32 residual chunk -> aT[rt] (bf16)
                for k in ks:
                    tp = ps_tp.tile([128, 128], F32, tag="tp", name="tp")
                    nc.tensor.transpose(
                        tp[:], h[rt][:, k * 128:(k + 1) * 128], ident_f[:])
                    nc.vector.tensor_copy(aT[rt][:, k, :], tp[:])

            def drain_add(rt, ch, pt, parts):
                # h[rt] chunk += psum; optionally compute the chunk's
                # sum-of-squares for the next norm while matmuls continue
                nc.vector.tensor_add(
                    h[rt][:, ch * 512:(ch + 1) * 512],
                    h[rt][:, ch * 512:(ch + 1) * 512], pt[:])
                if parts is not None:
                    junk = scr.tile([128, 512], BF16, tag="junk", name="jk5")
                    part = sm.tile([128, 1], F32, tag="sqp", name="sqp")
                    nc.scalar.activation(junk[:], h[rt][:, ch * 512:(ch + 1) * 512],
                                         func=ACTF.Square, accum_out=part[:])
                    parts[rt].append(part)

            dn_parts = None
            for t in range(STEPS):
                rstds = {}

                def qkv_phase():
                    # Q chunks first so attention can start mid-phase.
                    # Each weight chunk is loaded ONCE and used by both row
                    # tiles (halves the region's HBM demand, which would
                    # otherwise outrun DMA at ~590 GB/s and starve the PE).
                    order = (0, 1) if t == 0 else (2, 3, 4, 5, 0, 1)
                    for ch in order:
                        pq = [ps_mm.tile([128, 512], F32, tag="mm",
                                         name=f"pq{_rt}") for _rt in range(RT)]
                        for kg in range(4):
                            w = wts.tile([128, 4096], BF16, tag="w", name="wqkv")
                            nc.sync.dma_start(w[:, :2048], qkv_d[:][t, ch * 4 + kg])
                            for i in range(4):
                                k = kg * 4 + i
                                for rt in range(RT):
                                    nc.tensor.matmul(
                                        pq[rt][:], aT[rt][:, k, :],
                                        w[:, i * 512:(i + 1) * 512],
                                        start=(k == 0), stop=(k == KT - 1))
                        for rt in range(RT):
                            qkv_copy(rt, ch, pq[rt])

                def qkv_copy(rt, ch, pq):
                    # psum -> SBUF, rms-norm rstd folded into the scale
                    if ch == 0:
                        # store K rep-expanded to all heads: psum -> rep-0
                        # slice on ACT (scaled), then replicate to the other
                        # rep slots with small SBUF->SBUF DMAs
                        ke4 = Ke[t][rt][:].rearrange(
                            "p (kv r d) -> p kv r d", kv=KVH, r=REP)
                        nc.scalar.activation(
                            ke4[:, :, 0, :],
                            pq[:].rearrange("p (kv d) -> p kv d", kv=KVH),
                            func=ACTF.Copy, scale=rstds[rt][:])
                        for r in range(1, REP):
                            nc.sync.dma_start(ke4[:, :, r, :], ke4[:, :, 0, :])
                    elif ch == 1:
                        nc.scalar.activation(Vc[t][rt][:], pq[:],
                                             func=ACTF.Copy, scale=rstds[rt][:])
                    else:
                        hf, part = (ch - 2) // 2, (ch - 2) % 2
                        dst = Q[rt][hf][:, part * 512:(part + 1) * 512]
                        nc.scalar.activation(dst, pq[:], func=ACTF.Copy,
                                             scale=rstds[rt][:])

                def attn_rt(rt):
                    # t>=1 only (t=0 skips attention entirely via the wo0
                    # shortcut: softmax over one key == V)
                    eng = nc.gpsimd if GP_ATTN else nc.vector
                    ejs = {0: [], 1: []}
                    avs = {0: [], 1: []}
                    for hf in range(2):
                        k0, k1 = hf * 256, (hf + 1) * 256
                        oah = oa[rt][hf]
                        for j in range(t + 1):
                            ascr = asc.tile([128, 1024], ADT, tag="ascr",
                                            name="ascr")
                            nc.vector.tensor_tensor(
                                ascr[:], Q[rt][hf][:],
                                Ke[j][rt][:, hf * 1024:(hf + 1) * 1024],
                                op=ALU.mult)
                            sc = sm.tile([128, 16], F32, tag="sc", name="sc")
                            nc.vector.tensor_reduce(
                                sc[:],
                                ascr[:].rearrange("p (h d) -> p h d", h=16),
                                axis=AX.X, op=ALU.add)
                            ej = sm.tile([128, 16], F32, tag="ej", name="ej")
                            nc.scalar.activation(ej[:], sc[:], func=ACTF.Exp)
                            ejs[hf].append(ej)
                            if j == 0:
                                eng.tensor_tensor(
                                    _q4h(oah[:]), _hb4h(ej[:]),
                                    _kv4h(Vc[j][rt][:, k0:k1]), op=ALU.mult)
                            else:
                                av = avp.tile([128, 1024], ADT, tag="av",
                                              name="av")
                                eng.tensor_tensor(
                                    _q4h(av[:]), _hb4h(ej[:]),
                                    _kv4h(Vc[j][rt][:, k0:k1]), op=ALU.mult)
                                avs[hf].append(av)
                    for hf in range(2):
                        oah = oa[rt][hf]
                        den = sm.tile([128, 16], F32, tag="den", name="den")
                        nc.vector.tensor_add(den[:], ejs[hf][0][:], ejs[hf][1][:])
                        for ej in ejs[hf][2:]:
                            nc.vector.tensor_add(den[:], den[:], ej[:])
                        rec = sm.tile([128, 16], F32, tag="rec", name="rec")
                        nc.vector.reciprocal(rec[:], den[:])
                        for av in avs[hf]:
                            nc.vector.tensor_add(oah[:], oah[:], av[:])
                        nc.vector.tensor_tensor(
                            _q4h(oah[:]), _hb4h(rec[:]),
                            _q4h(oah[:]), op=ALU.mult)

                def o_tp_rt(rt):
                    if t == 0:
                        # aT := Vc^T (4 k-tiles); O uses the rep-summed wo0
                        for k in range(4):
                            tp = ps_tp.tile([128, 128], ADT, tag="tp", name="tp")
                            nc.tensor.transpose(
                                tp[:], Vc[0][rt][:, k * 128:(k + 1) * 128],
                                ident_b[:])
                            nc.vector.tensor_copy(aT[rt][:, k, :], tp[:])
                        return
                    for k in range(KT):
                        hf, kk = k // 8, k % 8
                        tp = ps_tp.tile([128, 128], ADT, tag="tp", name="tp")
                        nc.tensor.transpose(
                            tp[:], oa[rt][hf][:, kk * 128:(kk + 1) * 128],
                            ident_b[:])
                        nc.vector.tensor_copy(aT[rt][:, k, :], tp[:])

                def o_phase_t0(o_parts):
                    # t=0: oa == V, so O contracts V^T against rep-summed wo0;
                    # cheap enough to run joint over row tiles
                    for ch in range(4):
                        po = [ps_mm.tile([128, 512], F32, tag="mm",
                                         name=f"po{_rt}") for _rt in range(RT)]
                        w = wts.tile([128, 4096], BF16, tag="w", name="w")
                        nc.sync.dma_start(w[:, :2048], o0_d[:][ch])
                        for k in range(4):
                            for rt in range(RT):
                                nc.tensor.matmul(
                                    po[rt][:], aT[rt][:, k, :],
                                    w[:, k * 512:(k + 1) * 512],
                                    start=(k == 0), stop=(k == 3))
                        for rt in range(RT):
                            drain_add(rt, ch, po[rt], o_parts)

                def o_mm_rt(rt, po, drains=None):
                    # per-row-tile O so attn(rt1) hides under O(rt0) matmuls;
                    # weights double-loaded (affordable: QKV is shared now)
                    for ch in range(4):
                        po[ch] = ps_mm.tile([128, 512], F32, tag="mm",
                                            name=f"po{ch}")
                        for kg in range(4):
                            w = wts.tile([128, 4096], BF16, tag="w", name="w")
                            nc.sync.dma_start(w[:, :2048], o_d[:][ch * 4 + kg])
                            for i in range(4):
                                k = kg * 4 + i
                                nc.tensor.matmul(
                                    po[ch][:], aT[rt][:, k, :],
                                    w[:, i * 512:(i + 1) * 512],
                                    start=(k == 0), stop=(k == KT - 1))
                        if drains is not None:
                            drain_add(rt, ch, po[ch], drains)

                x2s = {}

                def norm2_mul(rt, o_parts):
                    rstd = rstd_of(rt, o_parts)
                    x = scr.tile([128, D], BF16, tag="scr", name=f"x2_{rt}")
                    # chunked so the transposes can pipeline behind it
                    for c in range(4):
                        nc.scalar.activation(x[:, c * 512:(c + 1) * 512],
                                             h[rt][:, c * 512:(c + 1) * 512],
                                             func=ACTF.Copy, scale=rstd[:])
                    x2s[rt] = x

                def norm2_tp(rt):
                    x = x2s[rt]
                    for k in range(KT):
                        tp = ps_tp.tile([128, 128], BF16, tag="tp", name="tp")
                        nc.tensor.transpose(
                            tp[:], x[:, k * 128:(k + 1) * 128], ident_b[:])
                        nc.vector.tensor_copy(
                            x2T[:, k, rt * 128:(rt + 1) * 128], tp[:])

                # ---- norm1 rstd + (transposes done during prev D) + QKV ----
                for rt in range(RT):
                    rstds[rt] = rstd_of(rt, dn_parts)
                    if t == 0:
                        tp_h(rt, range(KT))
                if t >= 1:
                    act_preload(ACTF.Exp)
                qkv_phase()

                # ---- attention + O-proj, row-tile staggered ----
                o_parts = [[] for _ in range(RT)]
                if t == 0:
                    o_tp_rt(0)
                    o_tp_rt(1)
                    o_phase_t0(o_parts)
                else:
                    po0, po1 = {}, {}
                    attn_rt(0)      # overlaps QKV's K/V chunk tail on PE
                    o_tp_rt(0)
                    o_mm_rt(0, po0)
                    attn_rt(1)      # overlaps O(rt0) matmuls on PE
                    for ch in range(4):
                        drain_add(0, ch, po0[ch], o_parts)
                    norm2_mul(0, o_parts)
                    o_tp_rt(1)
                    o_mm_rt(1, po1, drains=o_parts)
                if t == 0:
                    norm2_mul(0, o_parts)
                norm2_mul(1, o_parts)
                norm2_tp(0)
                norm2_tp(1)
                act_preload(ACTF.Sigmoid if SIM_SAFE else ACTF.Silu)

                # ---- MLP in two ff halves: gate/up -> mT, then down ----
                dn_parts = [[] for _ in range(RT)] if t < STEPS - 1 else None
                for half in range(2):
                    for pr in range(16 * half, 16 * (half + 1)):
                        wg = wts.tile([128, 4096], BF16, tag="w", name="wgt")
                        nc.sync.dma_start(wg[:], g_d[:][pr])
                        wu = wts.tile([128, 4096], BF16, tag="w", name="wut")
                        nc.sync.dma_start(wu[:], u_d[:][pr])
                        for mgi in range(2):
                            mloc = (pr * 2 + mgi) - 32 * half
                            pg = ps_gu.tile([128, R], F32, tag="gu", name="pg")
                            for k in range(KT):
                                c = (mgi * KT + k) * 128
                                nc.tensor.matmul(
                                    pg[:], wg[:, c:c + 128], x2T[:, k, :],
                                    start=(k == 0), stop=(k == KT - 1))
                            pu = ps_gu.tile([128, R], F32, tag="gu", name="pu")
                            for k in range(KT):
                                c = (mgi * KT + k) * 128
                                nc.tensor.matmul(
                                    pu[:], wu[:, c:c + 128], x2T[:, k, :],
                                    start=(k == 0), stop=(k == KT - 1))
                            sg = sm.tile([128, R], BF16, tag="sg", name="sg")
                            if SIM_SAFE:
                                # CoreSim lacks Silu; silu(x) = x * sigmoid(x)
                                nc.scalar.activation(sg[:], pg[:], func=ACTF.Sigmoid)
                                tmp = asc.tile([128, R], F32, tag="sgt", name="sgt")
                                nc.vector.tensor_tensor(
                                    tmp[:], sg[:], pg[:], op=ALU.mult)
                                nc.vector.tensor_tensor(
                                    mT[:, mloc, :], tmp[:], pu[:], op=ALU.mult)
                            else:
                                nc.scalar.activation(sg[:], pg[:], func=ACTF.Silu)
                                nc.vector.tensor_tensor(
                                    mT[:, mloc, :], sg[:], pu[:], op=ALU.mult)
                    for ch in range(4):
                        pd_ = [ps_mm.tile([128, 512], F32, tag="mm", name=f"pd{_rt}")
                               for _rt in range(RT)]
                        for kfg in range(4 * half, 4 * (half + 1)):
                            w = wts.tile([128, 4096], BF16, tag="w", name="wdt")
                            nc.sync.dma_start(w[:], d_d[:][ch * 8 + kfg])
                            for i in range(8):
                                kf = kfg * 8 + i
                                kfl = kf - 32 * half
                                for rt in range(RT):
                                    nc.tensor.matmul(
                                        pd_[rt][:],
                                        mT[:, kfl, rt * 128:(rt + 1) * 128],
                                        w[:, i * 512:(i + 1) * 512],
                                        start=(kf == 32 * half),
                                        stop=(kf == 32 * half + 31))
                        for rt in range(RT):
                            drain_add(rt, ch, pd_[rt],
                                      dn_parts if half == 1 else None)
                            if half == 1 and t < STEPS - 1:
                                # next step's norm1 transposes, interleaved
                                # with the remaining down matmuls
                                tp_h(rt, range(ch * 4, ch * 4 + 4))

                # ---- store step output ----
                for rt in range(RT):
                    nc.gpsimd.dma_start(out_d[:][t, rt], h[rt][:])

    nc.compile()
    return nc


# ---------------------------------------------------------------- host prep
def _rope_cs(t):
    inv = 1.0 / (THETA ** (np.arange(0, HD, 2, dtype=np.float64) / HD))
    emb = np.concatenate([t * inv, t * inv])
    return np.cos(emb), np.sin(emb)


def _rope_cols(w, t, nheads):
    # w: [D, nheads*HD] fp; returns rope'd version for position t
    w3 = w.reshape(D, nheads, HD)
    cos, sin = _rope_cs(t)
    wrot = np.concatenate([-w3[:, :, HD // 2:], w3[:, :, :HD // 2]], axis=2)
    return (w3 * cos[None, None, :] + wrot * sin[None, None, :]).reshape(D, nheads * HD)


def _pack_rhs(w, n_ch, n_kg):
    # w [K, n_ch*512]; chunks (ch, kg): [128, 4*512]; kg covers 4 k-tiles
    kt = w.shape[0] // 128
    A = w.reshape(n_kg, kt // n_kg, 128, n_ch, 512)
    return np.ascontiguousarray(A.transpose(3, 0, 2, 1, 4)).reshape(
        n_ch * n_kg, 128, (kt // n_kg) * 512)


def _pack_lhs_gu(w):
    # w [D, DFF] -> [32 pairs][128, (mgi 2, k 16, 128)]
    B = w.reshape(KT, 128, 32, 2, 128)
    return np.ascontiguousarray(B.transpose(2, 1, 3, 0, 4)).reshape(32, 128, 4096)


def _pack_rhs_dn(w):
    # w [DFF, D] -> chunks (ch 4, kfg 8): [128, (i 8, 512)]
    C = w.reshape(8, 8, 128, 4, 512)
    return np.ascontiguousarray(C.transpose(3, 0, 2, 1, 4)).reshape(32, 128, 4096)


def _gather_indices(comp_seq_lens, inst_lens):
    seqs = np.asarray(comp_seq_lens)
    insts = np.asarray(inst_lens)
    idx, off = [], 0
    for s, i in zip(seqs, insts):
        s, i = int(s), int(i)
        idx.append(np.arange(off + i - 1, off + s - 1))
        off += s
    return np.concatenate(idx)


def _prep_inputs(hidden_states, comp_seq_lens, inst_lens, w_q, w_k, w_v, w_o,
                 ln1_w, ln2_w, w_gate, w_up, w_down):
    idx = _gather_indices(comp_seq_lens, inst_lens)
    h0 = np.asarray(hidden_states, np.float32)[0, idx]          # [N, D]
    N = h0.shape[0]
    assert N == NCORES * R, f"expected {NCORES*R} rows, got {N}"

    ln1 = np.asarray(ln1_w, np.float64)
    ln2 = np.asarray(ln2_w, np.float64)
    wq_e = np.asarray(w_q, np.float64) * ln1[:, None] * (HD ** -0.5)
    wk_e = np.asarray(w_k, np.float64) * ln1[:, None]
    wv_e = np.asarray(w_v, np.float64) * ln1[:, None]
    wg_e = np.asarray(w_gate, np.float64) * ln2[:, None]
    wu_e = np.asarray(w_up, np.float64) * ln2[:, None]

    qkv_pack = np.empty((STEPS, 24, 128, 2048), NP_W)
    for t in range(STEPS):
        wq_t = _rope_cols(wq_e, t, HEADS)
        wk_t = _rope_cols(wk_e, t, KVH)
        qkv = np.concatenate([wk_t, wv_e, wq_t], axis=1)  # K, V, Q order
        qkv_pack[t] = _pack_rhs(qkv, 6, 4).astype(NP_W)

    # t=0 shortcut: attention output == V (rep-broadcast), so O-proj
    # contracts V [*,512] against the rep-summed Wo rows
    wo64 = np.asarray(w_o, np.float64)
    wo0 = wo64.reshape(KVH, REP, HD, D).sum(axis=1).reshape(KVH * HD, D)

    weights = {
        "wqkv": qkv_pack,
        "wo": _pack_rhs(wo64, 4, 4).astype(NP_W),
        "wo0": _pack_rhs(wo0, 4, 1).astype(NP_W),
        "wg": _pack_lhs_gu(wg_e).astype(NP_W),
        "wu": _pack_lhs_gu(wu_e).astype(NP_W),
        "wd": _pack_rhs_dn(np.asarray(w_down, np.float64)).astype(NP_W),
    }
    h0_cores = h0.reshape(NCORES, RT, 128, D)
    return weights, h0_cores


def kernel(**inputs):
    weights, h0_cores = _prep_inputs(**inputs)

    if "nc" not in _CACHE:
        _CACHE["nc"] = _build_program()
    nc = _CACHE["nc"]

    in_maps = [dict(weights, h0=np.ascontiguousarray(h0_cores[c]))
               for c in range(NCORES)]
    res = run_bass_kernel_spmd(nc, in_maps, core_ids=list(range(NCORES)))
    _CACHE["last_results"] = res

    outs = []
    for c in range(NCORES):
        o = res.results[c]["out"]                  # [5, RT, 128, D]
        outs.append(o.reshape(STEPS, R, D).transpose(1, 0, 2))
    return np.concatenate(outs, axis=0)            # [N, 5, D]
